# revision 7
# baseline (speedup 1.0000x reference)
"""Trainium2 Bass kernel for 2-layer GNN message passing (CHARM).

Strategy (per the edge-sharding hint):
  - Edges are sharded across 8 NeuronCores, sorted by destination node, and
    padded into 128-edge chunks aligned to 128-node destination tiles (same
    chunk structure on every core so one SPMD program serves all 8).
  - msg1 is split: the node-dependent half (h @ W1a) is computed once per
    node as P1 and gathered per edge via dma_gather; the edge-feature half
    ((edge_attr|edge_mark|1) @ (W1b|b1)) is a K=19 matmul per chunk.
  - msg2 is linear, so it commutes with segment_sum:
    aggr = segment_sum(relu(m1)/deg) @ W2 (+ b2 folded into the update bias).
  - segment_sum is a one-hot selection matmul accumulated in PSUM per node
    tile; the partial aggregates A are AllReduce'd across cores.
  - The node-update MLP runs replicated (feature-major) on every core.
"""

import math
import os
import sys
import types

import numpy as np

N = 10000
E = 320000
NP = 10240          # padded node count (80 tiles of 128)
NCORES = 8
HID = 128
TILE = 128
NT = NP // TILE     # 80 node tiles
ECORE = E // NCORES
GBLK = 8            # gather block size in chunks (8*128 = 1024 idxs; >1024 idxs per dma_gather crashes the Q7 DGE ring)


def _prep(x, edge_index, edge_attr, edge_mark, deg_out, params):
    """Host-side sharding/layout prep. Returns (in_maps, meta)."""
    ei = np.asarray(edge_index)
    src_all = ei[0].astype(np.int64)
    dst_all = ei[1].astype(np.int64)
    ea = np.asarray(edge_attr, dtype=np.float32)
    em = np.asarray(edge_mark, dtype=np.float32)
    deg_out = np.asarray(deg_out, dtype=np.float32)
    x = np.asarray(x, dtype=np.float32)

    deg = np.where(deg_out == 0, 1.0, deg_out).astype(np.float32)
    invdeg = (1.0 / deg).astype(np.float32)
    has_isolated = bool((deg_out == 0).any())
    mask_row = (deg_out > 0).astype(np.float32)
    mask_pad = np.zeros((1, NP), np.float32)
    mask_pad[0, :N] = mask_row

    # per-core dst-sorted order and per-(core, tile) counts
    orders = []
    cnts = np.zeros((NCORES, NT), np.int64)
    for c in range(NCORES):
        sl = slice(c * ECORE, (c + 1) * ECORE)
        o = np.argsort(dst_all[sl], kind="stable") + c * ECORE
        orders.append(o)
        cnts[c] = np.bincount(dst_all[o] // TILE, minlength=NT)
    nch = np.maximum(np.ceil(cnts / TILE).astype(np.int64).max(axis=0), 1)
    t_total = int(nch.sum())
    epad = t_total * TILE
    chunk_tile = np.repeat(np.arange(NT), nch)  # [t_total]

    # shared constants
    ident = np.eye(128, dtype=np.float32)
    iota = np.tile(np.arange(128, dtype=np.float32)[None, :], (128, 1))
    x_fm = np.zeros((128, NP), np.float32)
    x_fm[:, :N] = x.T

    p = params
    lw = []
    for lp in p["layers"]:
        W1 = np.asarray(lp["msg1"]["w"], np.float32)
        b1 = np.asarray(lp["msg1"]["b"], np.float32)
        W2 = np.asarray(lp["msg2"]["w"], np.float32)
        b2 = np.asarray(lp["msg2"]["b"], np.float32)
        Wu1 = np.asarray(lp["up1"]["w"], np.float32)
        bu1 = np.asarray(lp["up1"]["b"], np.float32)
        Wu2 = np.asarray(lp["up2"]["w"], np.float32)
        bu2 = np.asarray(lp["up2"]["b"], np.float32)
        w1b_aug = np.concatenate([W1[HID:], b1[None, :]], axis=0)  # [19,128]
        lw.append(dict(
            w1a=W1[:HID],                   # [128,128] lhsT for P1
            w1b_aug=w1b_aug,                # [19,128]
            w2t=np.ascontiguousarray(W2.T),  # [128,128] lhsT for W2up
            wu1_top=Wu1[:HID],              # [128,128]
            wu1_bot=Wu1[HID:],              # [128,128]
            bu1_eff=(bu1 + (0.0 if has_isolated else 1.0) * (b2 @ Wu1[HID:]))
            .astype(np.float32).reshape(128, 1),
            c_row=(b2 @ Wu1[HID:]).astype(np.float32).reshape(1, 128),
            wu2=Wu2,
            bu2=bu2.reshape(128, 1),
        ))

    common = dict(
        x_fm=x_fm,
        ident=ident,
        iota=iota,
        mask=mask_pad,
        win=np.asarray(p["in_proj"]["w"], np.float32),
        bin=np.asarray(p["in_proj"]["b"], np.float32).reshape(128, 1),
        wp1=np.asarray(p["pred1"]["w"], np.float32),
        bp1=np.asarray(p["pred1"]["b"], np.float32).reshape(64, 1),
        wp2=np.asarray(p["pred2"]["w"], np.float32),
        bp2=np.asarray(p["pred2"]["b"], np.float32).reshape(1, 1),
    )
    for li, d in enumerate(lw):
        for k, v in d.items():
            common[f"l{li}_{k}"] = v

    in_maps = []
    for c in range(NCORES):
        o = orders[c]
        src = src_all[o]
        dst = dst_all[o]
        eaem = np.concatenate([ea[o], em[o], np.ones((ECORE, 1), np.float32)],
                              axis=1)  # [ECORE, 19]
        src_p = np.zeros(epad, np.int64)
        eaem_p = np.zeros((epad, 19), np.float32)
        dstrel = np.full(epad, -1.0, np.float32)
        ivd = np.zeros(epad, np.float32)
        pos = 0
        epos = 0
        for t in range(NT):
            cnt = int(cnts[c, t])
            seg = slice(epos, epos + cnt)
            src_p[pos:pos + cnt] = src[seg]
            eaem_p[pos:pos + cnt] = eaem[seg]
            dstrel[pos:pos + cnt] = dst[seg] - t * TILE
            ivd[pos:pos + cnt] = invdeg[dst[seg]]
            epos += cnt
            pos += int(nch[t]) * TILE
        # dma_gather idx layout: idx j -> partition j%16, col j//16, x8 replicas
        gidx = np.tile(src_p.astype(np.int16).reshape(epad // 16, 16).T, (8, 1))
        m = dict(common)
        m["gidx"] = np.ascontiguousarray(gidx)
        m["eaemT"] = np.ascontiguousarray(eaem_p.T)          # [19, epad]
        m["dstrel"] = np.ascontiguousarray(
            dstrel.reshape(t_total, 128).T)                   # [128, t_total]
        m["ivd"] = np.ascontiguousarray(
            ivd.reshape(t_total, 128).T)                      # [128, t_total]
        in_maps.append(m)

    meta = dict(t_total=t_total, epad=epad, chunk_tile=chunk_tile.tolist(),
                has_isolated=has_isolated)
    return in_maps, meta


def _build(meta):
    import concourse.bacc as bacc
    import concourse.tile as tile
    from concourse import library_config, mybir
    from concourse._compat import get_trn_type

    f32 = mybir.dt.float32
    i16 = mybir.dt.int16
    Relu = mybir.ActivationFunctionType.Relu
    Copy = mybir.ActivationFunctionType.Copy

    t_total = meta["t_total"]
    epad = meta["epad"]
    chunk_tile = meta["chunk_tile"]
    has_isolated = meta["has_isolated"]

    nc = bacc.Bacc(get_trn_type() or "TRN2")

    def din(name, shape, dt=f32):
        return nc.dram_tensor(name, shape, dt, kind="ExternalInput")

    x_fm = din("x_fm", [128, NP])
    gidx = din("gidx", [128, epad // 16], i16)
    eaemT = din("eaemT", [19, epad])
    dstrel = din("dstrel", [128, t_total])
    ivd = din("ivd", [128, t_total])
    ident = din("ident", [128, 128])
    iota = din("iota", [128, 128])
    mask = din("mask", [1, NP])
    win = din("win", [128, 128])
    bin_ = din("bin", [128, 1])
    wp1 = din("wp1", [128, 64])
    bp1 = din("bp1", [64, 1])
    wp2 = din("wp2", [64, 1])
    bp2 = din("bp2", [1, 1])
    L = []
    for li in range(2):
        L.append({k: din(f"l{li}_{k}", shp) for k, shp in [
            ("w1a", [128, 128]), ("w1b_aug", [19, 128]), ("w2t", [128, 128]),
            ("wu1_top", [128, 128]), ("wu1_bot", [128, 128]),
            ("bu1_eff", [128, 1]), ("c_row", [1, 128]),
            ("wu2", [128, 128]), ("bu2", [128, 1])]})
    y_out = nc.dram_tensor("y", [1, NP], f32, kind="ExternalOutput")

    NCH = NP // 512  # 20 node-phase column chunks

    with tile.TileContext(nc) as tc:
        nc.gpsimd.load_library(library_config.mlp)
        with (
            tc.tile_pool(name="const", bufs=1) as cp,
            tc.tile_pool(name="stream", bufs=3) as sp,
            tc.tile_pool(name="work", bufs=3) as wp,
            tc.tile_pool(name="pgp", bufs=2) as pgp,
            tc.tile_pool(name="ep", bufs=2) as ep,
            tc.tile_pool(name="big", bufs=1) as bp,
            tc.tile_pool(name="psum", bufs=2, space="PSUM") as pp,
            tc.tile_pool(name="dram", bufs=2, space="DRAM") as dp,
        ):
            def load_const(ap, shape, dt=f32, tag=None):
                t = cp.tile(shape, dt, tag=tag)
                nc.sync.dma_start(t[:], ap[:])
                return t

            c_gidx = load_const(gidx, [128, epad // 16], i16, tag="c_gidx")
            c_dstrel = load_const(dstrel, [128, t_total], tag="c_dstrel")
            c_ivd = load_const(ivd, [128, t_total], tag="c_ivd")
            c_ident = load_const(ident, [128, 128], tag="c_ident")
            c_iota = load_const(iota, [128, 128], tag="c_iota")
            c_mask = (load_const(mask, [1, NP], tag="c_mask")
                      if has_isolated else None)
            c_win = load_const(win, [128, 128], tag="c_win")
            c_bin = load_const(bin_, [128, 1], tag="c_bin")
            c_wp1 = load_const(wp1, [128, 64], tag="c_wp1")
            c_bp1 = load_const(bp1, [64, 1], tag="c_bp1")
            c_wp2 = load_const(wp2, [64, 1], tag="c_wp2")
            c_bp2 = load_const(bp2, [1, 1], tag="c_bp2")
            cl = []
            for li in range(2):
                cl.append({k: load_const(L[li][k], L[li][k].shape,
                                         tag=f"c_l{li}_{k}")
                           for k in L[li]})

            h_fm = bp.tile([128, NP], f32)  # resident node features

            # ---- in_proj: h = relu(Win.T @ x + bin) ----
            for j in range(NCH):
                cols = slice(j * 512, (j + 1) * 512)
                xt = sp.tile([128, 512], f32, tag="xin")
                nc.sync.dma_start(xt[:], x_fm[:, cols])
                ps = pp.tile([128, 512], f32, space="PSUM", tag="q")
                nc.tensor.matmul(ps[:], c_win[:], xt[:], start=True, stop=True)
                nc.scalar.activation(h_fm[:, cols], ps[:], Relu, bias=c_bin[:])

            for li in range(2):
                w = cl[li]
                # ---- P1 = h.T @ W1a  (node-major, [NP, 128] in DRAM) ----
                p1 = dp.tile([NP, 128], f32, tag="p1")
                for t in range(NT):
                    ncols = slice(t * 128, (t + 1) * 128)
                    ps = pp.tile([128, 128], f32, space="PSUM", tag="q")
                    nc.tensor.matmul(ps[:], h_fm[:, ncols], w["w1a"][:],
                                     start=True, stop=True)
                    st = wp.tile([128, 128], f32, tag="p1st")
                    nc.vector.tensor_copy(st[:], ps[:])
                    nc.sync.dma_start(p1[ncols, :], st[:])

                # ---- edge phase ----
                a_dram = dp.tile([128, NP], f32, tag="a_in")
                a_red = dp.tile([128, NP], f32, tag="a_out")
                nblocks = math.ceil(t_total / GBLK)
                a_ps = None
                for b in range(nblocks):
                    c0 = b * GBLK
                    bch = min(GBLK, t_total - c0)
                    pg = pgp.tile([128, bch, 128], f32, tag="pg")
                    if os.environ.get("K_NO_GATHER"):
                        nc.gpsimd.memset(pg[:], 0)
                    else:
                        nc.gpsimd.dma_gather(
                            pg[:], p1[:],
                            c_gidx[:, c0 * 8:(c0 + bch) * 8],
                            bch * 128, bch * 128, 128,
                        )
                    et = ep.tile([19, bch * 128], f32, tag="eaem")
                    nc.sync.dma_start(et[:], eaemT[:, c0 * 128:(c0 + bch) * 128])
                    for k in range(bch):
                        ci = c0 + k
                        t = chunk_tile[ci]
                        q = pp.tile([128, 128], f32, space="PSUM", tag="q")
                        nc.tensor.matmul(q[:], et[:, k * 128:(k + 1) * 128],
                                         w["w1b_aug"][:], start=True, stop=False)
                        nc.tensor.matmul(q[:], c_ident[:], pg[:, k, :],
                                         start=False, stop=True)
                        m1n = wp.tile([128, 128], f32, tag="m1n")
                        nc.scalar.activation(m1n[:], q[:], Relu,
                                             scale=c_ivd[:, ci:ci + 1])
                        sel = wp.tile([128, 128], f32, tag="sel")
                        nc.vector.tensor_tensor(
                            out=sel[:],
                            in0=c_dstrel[:, ci:ci + 1].to_broadcast([128, 128]),
                            in1=c_iota[:],
                            op=mybir.AluOpType.is_equal,
                        )
                        first = ci == 0 or chunk_tile[ci - 1] != t
                        last = ci == t_total - 1 or chunk_tile[ci + 1] != t
                        if first:
                            a_ps = pp.tile([128, 128], f32, space="PSUM",
                                           tag="a")
                        nc.tensor.matmul(a_ps[:], m1n[:], sel[:],
                                         start=first, stop=last)
                        if last:
                            ast = wp.tile([128, 128], f32, tag="ast")
                            nc.vector.tensor_copy(ast[:], a_ps[:])
                            nc.sync.dma_start(
                                a_dram[:, t * 128:(t + 1) * 128], ast[:])

                if os.environ.get("K_NO_COLL"):
                    nc.gpsimd.dma_start(a_red[:], a_dram[:])
                else:
                    nc.gpsimd.collective_compute(
                        "AllReduce",
                        mybir.AluOpType.add,
                        replica_groups=[list(range(NCORES))],
                        ins=[a_dram.opt()],
                        outs=[a_red.opt()],
                    )

                # ---- node phase (feature-major, replicated) ----
                # W2up = W2 @ Wu1_bot  (via lhsT = W2.T)
                ps = pp.tile([128, 128], f32, space="PSUM", tag="q")
                nc.tensor.matmul(ps[:], w["w2t"][:], w["wu1_bot"][:],
                                 start=True, stop=True)
                w2up = wp.tile([128, 128], f32, tag="w2up")
                nc.vector.tensor_copy(w2up[:], ps[:])
                for j in range(NCH):
                    cols = slice(j * 512, (j + 1) * 512)
                    at = sp.tile([128, 512], f32, tag="ain")
                    nc.sync.dma_start(at[:], a_red[:, cols])
                    u1 = pp.tile([128, 512], f32, space="PSUM", tag="q")
                    nc.tensor.matmul(u1[:], w["wu1_top"][:], h_fm[:, cols],
                                     start=True, stop=False)
                    nc.tensor.matmul(u1[:], w2up[:], at[:],
                                     start=False, stop=not has_isolated)
                    if has_isolated:
                        nc.tensor.matmul(u1[:], w["c_row"][:], c_mask[:, cols],
                                         start=False, stop=True)
                    z1 = wp.tile([128, 512], f32, tag="z1")
                    nc.scalar.activation(z1[:], u1[:], Relu,
                                         bias=w["bu1_eff"][:])
                    u2 = pp.tile([128, 512], f32, space="PSUM", tag="a")
                    nc.tensor.matmul(u2[:], w["wu2"][:], z1[:],
                                     start=True, stop=True)
                    hn = wp.tile([128, 512], f32, tag="hn")
                    nc.vector.tensor_add(hn[:], h_fm[:, cols], u2[:])
                    nc.scalar.activation(h_fm[:, cols], hn[:], Relu,
                                         bias=w["bu2"][:])

            # ---- prediction head ----
            for j in range(NCH):
                cols = slice(j * 512, (j + 1) * 512)
                zp_ps = pp.tile([64, 512], f32, space="PSUM", tag="q")
                nc.tensor.matmul(zp_ps[:], c_wp1[:], h_fm[:, cols],
                                 start=True, stop=True)
                zp = wp.tile([64, 512], f32, tag="zp")
                nc.scalar.activation(zp[:], zp_ps[:], Relu, bias=c_bp1[:])
                y_ps = pp.tile([1, 512], f32, space="PSUM", tag="a")
                nc.tensor.matmul(y_ps[:], c_wp2[:], zp[:],
                                 start=True, stop=True)
                yt = wp.tile([1, 512], f32, tag="yt")
                nc.vector.tensor_scalar_add(yt[:], y_ps[:], c_bp2[:])
                nc.sync.dma_start(y_out[:, cols], yt[:])

    nc.compile()
    return nc


_CACHE = {}


def _get_program(meta_key, meta):
    if meta_key not in _CACHE:
        _CACHE[meta_key] = _build(meta)
    return _CACHE[meta_key]


def kernel(x, edge_index, edge_attr, edge_mark, deg_out, params,
           _trace=False):
    from concourse.bass_utils import run_bass_kernel_spmd

    in_maps, meta = _prep(x, edge_index, edge_attr, edge_mark, deg_out, params)
    meta_key = (meta["t_total"], tuple(meta["chunk_tile"]),
                meta["has_isolated"])
    nc = _get_program(meta_key, meta)

    kwargs = {}
    if _trace:
        # antenv.axon_hooks is absent in this image; provide the ctypes hook.
        try:
            import antenv.axon_hooks  # noqa: F401
        except ImportError:
            from trn_agent_boot.trn_boot import _ntff_profile_via_ctypes
            m = types.ModuleType("antenv.axon_hooks")
            m.get_axon_ntff_profile_hook = lambda: _ntff_profile_via_ctypes(
                "/opt/axon/libaxon_pjrt.so")
            sys.modules["antenv.axon_hooks"] = m
        kwargs["trace"] = True

    res = run_bass_kernel_spmd(nc, in_maps, list(range(NCORES)), **kwargs)
    y = np.asarray(res.results[0]["y"]).reshape(-1)[:N].astype(np.float32)
    if _trace:
        kernel._last_exec_time_ns = res.exec_time_ns
        kernel._last_results = res
    return y


# revision 11
# speedup vs baseline: 1.6226x; 1.6226x over previous
"""Trainium2 Bass kernel for 2-layer GNN message passing (CHARM).

Strategy (per the edge-sharding hint):
  - Edges are dealt round-robin across 8 NeuronCores from a global
    dst-sorted order, padded into 128-edge chunks aligned to 128-node
    destination tiles (same chunk structure on every core -> one SPMD
    program serves all 8).
  - msg1 is split: the node-dependent half (h @ W1a) is computed once per
    node as P1 (bf16) and gathered per edge via dma_gather; the
    edge-feature half ((edge_attr|edge_mark|1) @ (W1b|b1)) is a K=19 bf16
    matmul per chunk, accumulated with the gathered P1 in fp32 PSUM.
  - msg2 is linear, so it commutes with segment_sum:
    aggr = segment_sum(relu(m1)/deg) @ W2 (+ b2 folded into the update
    bias). The 1/deg is folded into the one-hot selection matrix.
  - segment_sum is a one-hot selection matmul accumulated in PSUM per node
    tile; partial aggregates A are AllReduce'd (fp32) in two halves so the
    first AllReduce overlaps the second half's edge compute.
  - The node-update MLP runs replicated (feature-major, fp32) per core.
"""

import math
import os
import sys
import types

import numpy as np

N = 10000
E = 320000
NP = 10240          # padded node count (80 tiles of 128)
NCORES = 8
HID = 128
TILE = 128
NT = NP // TILE     # 80 node tiles
ECORE = E // NCORES
GBLK = 8            # chunks per dma_gather (1024 idxs; >1024 crashes DGE ring)
NQ = 4              # SWDGE queues for gather descriptor generation
HALF = NT // 2      # node tiles per AllReduce half


def _prep(x, edge_index, edge_attr, edge_mark, deg_out, params):
    """Host-side sharding/layout prep. Returns (in_maps, meta)."""
    import ml_dtypes

    bf16 = ml_dtypes.bfloat16
    ei = np.asarray(edge_index)
    src_all = ei[0].astype(np.int64)
    dst_all = ei[1].astype(np.int64)
    ea = np.asarray(edge_attr, dtype=np.float32)
    em = np.asarray(edge_mark, dtype=np.float32)
    deg_out = np.asarray(deg_out, dtype=np.float32)
    x = np.asarray(x, dtype=np.float32)

    deg = np.where(deg_out == 0, 1.0, deg_out).astype(np.float32)
    invdeg = (1.0 / deg).astype(np.float32)
    has_isolated = bool((deg_out == 0).any())
    mask_pad = np.zeros((1, NP), np.float32)
    mask_pad[0, :N] = (deg_out > 0).astype(np.float32)

    # global dst-sort, then deal round-robin -> per-(core,tile) counts +-1
    gorder = np.argsort(dst_all, kind="stable")
    orders = [gorder[c::NCORES] for c in range(NCORES)]
    cnts = np.zeros((NCORES, NT), np.int64)
    for c in range(NCORES):
        cnts[c] = np.bincount(dst_all[orders[c]] // TILE, minlength=NT)
    nch = np.maximum(np.ceil(cnts / TILE).astype(np.int64).max(axis=0), 1)
    t_total = int(nch.sum())
    epad = t_total * TILE
    chunk_tile = np.repeat(np.arange(NT), nch)  # [t_total]

    ident = np.eye(128, dtype=np.float32)
    iota = np.tile(np.arange(128, dtype=np.float32)[None, :], (128, 1))
    x_fm = np.zeros((128, NP), np.float32)
    x_fm[:, :N] = x.T

    p = params
    common = dict(
        x_fm=x_fm,
        ident=ident.astype(bf16),
        iota=iota.astype(bf16),
        mask=mask_pad,
        win=np.asarray(p["in_proj"]["w"], np.float32),
        bin=np.asarray(p["in_proj"]["b"], np.float32).reshape(128, 1),
        wp1=np.asarray(p["pred1"]["w"], np.float32),
        bp1=np.asarray(p["pred1"]["b"], np.float32).reshape(64, 1),
        wp2=np.asarray(p["pred2"]["w"], np.float32),
        bp2=np.asarray(p["pred2"]["b"], np.float32).reshape(1, 1),
    )
    for li, lp in enumerate(p["layers"]):
        W1 = np.asarray(lp["msg1"]["w"], np.float32)
        b1 = np.asarray(lp["msg1"]["b"], np.float32)
        W2 = np.asarray(lp["msg2"]["w"], np.float32)
        b2 = np.asarray(lp["msg2"]["b"], np.float32)
        Wu1 = np.asarray(lp["up1"]["w"], np.float32)
        bu1 = np.asarray(lp["up1"]["b"], np.float32)
        Wu2 = np.asarray(lp["up2"]["w"], np.float32)
        bu2 = np.asarray(lp["up2"]["b"], np.float32)
        d = dict(
            w1a=W1[:HID].astype(bf16),
            w1b_aug=np.concatenate([W1[HID:], b1[None, :]], 0).astype(bf16),
            w2t=np.ascontiguousarray(W2.T),
            wu1_top=Wu1[:HID],
            wu1_bot=Wu1[HID:],
            bu1_eff=(bu1 + (0.0 if has_isolated else 1.0) * (b2 @ Wu1[HID:]))
            .astype(np.float32).reshape(128, 1),
            c_row=(b2 @ Wu1[HID:]).astype(np.float32).reshape(1, 128),
            wu2=Wu2,
            bu2=bu2.reshape(128, 1),
        )
        for k, v in d.items():
            common[f"l{li}_{k}"] = v

    in_maps = []
    for c in range(NCORES):
        o = orders[c]
        src = src_all[o]
        dst = dst_all[o]
        ecn = len(o)
        eaem = np.concatenate([ea[o], em[o], np.ones((ecn, 1), np.float32)], 1)
        src_p = np.zeros(epad, np.int64)
        eaem_p = np.zeros((epad, 19), np.float32)
        dstrel = np.full(epad, -1.0, np.float32)
        ivd = np.zeros(epad, np.float32)
        pos = 0
        epos = 0
        for t in range(NT):
            cnt = int(cnts[c, t])
            seg = slice(epos, epos + cnt)
            src_p[pos:pos + cnt] = src[seg]
            eaem_p[pos:pos + cnt] = eaem[seg]
            dstrel[pos:pos + cnt] = dst[seg] - t * TILE
            ivd[pos:pos + cnt] = invdeg[dst[seg]]
            epos += cnt
            pos += int(nch[t]) * TILE
        # dma_gather idx layout: idx j -> partition j%16, col j//16, x8
        gidx = np.tile(src_p.astype(np.int16).reshape(epad // 16, 16).T, (8, 1))
        m = dict(common)
        m["gidx"] = np.ascontiguousarray(gidx)
        m["eaemT"] = np.ascontiguousarray(eaem_p.T).astype(bf16)  # [19, epad]
        m["dstrel"] = np.ascontiguousarray(
            dstrel.reshape(t_total, 128).T)                       # [128, T]
        m["ivd"] = np.ascontiguousarray(
            ivd.reshape(t_total, 128).T)                          # [128, T]
        in_maps.append(m)

    meta = dict(t_total=t_total, epad=epad, chunk_tile=chunk_tile.tolist(),
                has_isolated=has_isolated)
    return in_maps, meta


def _build(meta):
    import concourse.bacc as bacc
    import concourse.tile as tile
    from concourse import library_config, mybir
    from concourse._compat import get_trn_type

    f32 = mybir.dt.float32
    b16 = mybir.dt.bfloat16
    i16 = mybir.dt.int16
    Relu = mybir.ActivationFunctionType.Relu

    t_total = meta["t_total"]
    epad = meta["epad"]
    chunk_tile = meta["chunk_tile"]
    has_isolated = meta["has_isolated"]

    nc = bacc.Bacc(get_trn_type() or "TRN2", num_swdge_queues=NQ)

    def din(name, shape, dt=f32):
        return nc.dram_tensor(name, shape, dt, kind="ExternalInput")

    x_fm = din("x_fm", [128, NP])
    gidx = din("gidx", [128, epad // 16], i16)
    eaemT = din("eaemT", [19, epad], b16)
    dstrel = din("dstrel", [128, t_total])
    ivd = din("ivd", [128, t_total])
    ident = din("ident", [128, 128], b16)
    iota = din("iota", [128, 128], b16)
    mask = din("mask", [1, NP])
    win = din("win", [128, 128])
    bin_ = din("bin", [128, 1])
    wp1 = din("wp1", [128, 64])
    bp1 = din("bp1", [64, 1])
    wp2 = din("wp2", [64, 1])
    bp2 = din("bp2", [1, 1])
    L = []
    for li in range(2):
        L.append({k: din(f"l{li}_{k}", shp, dt) for k, shp, dt in [
            ("w1a", [128, 128], b16), ("w1b_aug", [19, 128], b16),
            ("w2t", [128, 128], f32),
            ("wu1_top", [128, 128], f32), ("wu1_bot", [128, 128], f32),
            ("bu1_eff", [128, 1], f32), ("c_row", [1, 128], f32),
            ("wu2", [128, 128], f32), ("bu2", [128, 1], f32)]})
    y_out = nc.dram_tensor("y", [1, NP], f32, kind="ExternalOutput")

    NCH = NP // 512  # 20 node-phase column chunks

    with tile.TileContext(nc) as tc:
        nc.gpsimd.load_library(library_config.mlp)
        with (
            tc.tile_pool(name="const", bufs=1) as cp,
            tc.tile_pool(name="stream", bufs=3) as sp,
            tc.tile_pool(name="work", bufs=3) as wp,
            tc.tile_pool(name="pgp", bufs=3) as pgp,
            tc.tile_pool(name="ep", bufs=3) as ep,
            tc.tile_pool(name="big", bufs=1) as bp,
            tc.tile_pool(name="psum", bufs=3, space="PSUM") as pp,
            tc.tile_pool(name="dram", bufs=2, space="DRAM") as dp,
        ):
            def load_const(ap, tag, dt=f32):
                t = cp.tile(ap.shape, dt, tag=tag)
                nc.sync.dma_start(t[:], ap[:])
                return t

            c_gidx = load_const(gidx, "c_gidx", i16)
            c_dstrel = load_const(dstrel, "c_dstrel")
            c_ivd = load_const(ivd, "c_ivd")
            c_ident = load_const(ident, "c_ident", b16)
            c_iota = load_const(iota, "c_iota", b16)
            c_mask = load_const(mask, "c_mask") if has_isolated else None
            c_win = load_const(win, "c_win")
            c_bin = load_const(bin_, "c_bin")
            c_wp1 = load_const(wp1, "c_wp1")
            c_bp1 = load_const(bp1, "c_bp1")
            c_wp2 = load_const(wp2, "c_wp2")
            c_bp2 = load_const(bp2, "c_bp2")
            cl = []
            for li in range(2):
                cl.append({
                    k: load_const(L[li][k], f"c_l{li}_{k}", L[li][k].dtype)
                    for k in L[li]})

            h_fm = bp.tile([128, NP], f32)      # resident features (fp32)
            h_bf = bp.tile([128, NP], b16)      # bf16 shadow for matmul lhsT

            # ---- in_proj: h = relu(Win.T @ x + bin) ----
            for j in range(NCH):
                cols = slice(j * 512, (j + 1) * 512)
                xt = sp.tile([128, 512], f32, tag="xin")
                nc.sync.dma_start(xt[:], x_fm[:, cols])
                ps = pp.tile([128, 512], f32, space="PSUM", tag="q")
                nc.tensor.matmul(ps[:], c_win[:], xt[:], start=True, stop=True)
                nc.scalar.activation(h_fm[:, cols], ps[:], Relu, bias=c_bin[:])
                nc.vector.tensor_copy(h_bf[:, cols], h_fm[:, cols])

            for li in range(2):
                w = cl[li]
                # ---- P1 = (h_bf.T @ W1a) in bf16, node-major in DRAM ----
                p1 = dp.tile([NP, 128], b16, tag="p1")
                for t in range(NT):
                    ncols = slice(t * 128, (t + 1) * 128)
                    ps = pp.tile([128, 128], f32, space="PSUM", tag="p1ps", bufs=2)
                    nc.tensor.matmul(ps[:], h_bf[:, ncols], w["w1a"][:],
                                     start=True, stop=True)
                    st = wp.tile([128, 128], b16, tag="p1st")
                    nc.vector.tensor_copy(st[:], ps[:])
                    nc.sync.dma_start(p1[ncols, :], st[:])

                # ---- edge phase ----
                a_h = [dp.tile([128, HALF * 128], f32, tag=f"a_in{h}",
                               name=f"a_in{h}_l{li}")
                       for h in range(2)]
                a_r = [dp.tile([128, HALF * 128], f32, tag=f"a_out{h}",
                               name=f"a_out{h}_l{li}")
                       for h in range(2)]
                nblocks = math.ceil(t_total / GBLK)
                a_ps = None
                for b in range(nblocks):
                    c0 = b * GBLK
                    bch = min(GBLK, t_total - c0)
                    pg = pgp.tile([128, bch, 128], b16, tag="pg")
                    nc.gpsimd.dma_gather(
                        pg[:], p1[:],
                        c_gidx[:, c0 * 8:(c0 + bch) * 8],
                        bch * 128, bch * 128, 128,
                        queue_num=b % NQ,
                    )
                    et = ep.tile([19, bch * 128], b16, tag="eaem")
                    nc.sync.dma_start(et[:],
                                      eaemT[:, c0 * 128:(c0 + bch) * 128])
                    for g0 in range(0, bch, 4):
                        gsz = min(4, bch - g0)
                        q = pp.tile([128, 512], f32, space="PSUM", tag="q")
                        for k in range(g0, g0 + gsz):
                            qs = slice((k - g0) * 128, (k - g0 + 1) * 128)
                            nc.tensor.matmul(
                                q[:, qs], et[:, k * 128:(k + 1) * 128],
                                w["w1b_aug"][:], start=True, stop=False)
                            nc.tensor.matmul(
                                q[:, qs], c_ident[:], pg[:, k, :],
                                start=False, stop=True)
                        m1n = wp.tile([128, 512], b16, tag="m1n")
                        nc.scalar.activation(m1n[:, :gsz * 128],
                                             q[:, :gsz * 128], Relu)
                        for k in range(g0, g0 + gsz):
                            ci = c0 + k
                            t = chunk_tile[ci]
                            sel = wp.tile([128, 128], b16, tag="sel")
                            nc.vector.tensor_scalar(
                                out=sel[:], in0=c_iota[:],
                                scalar1=c_dstrel[:, ci:ci + 1],
                                scalar2=c_ivd[:, ci:ci + 1],
                                op0=mybir.AluOpType.is_equal,
                                op1=mybir.AluOpType.mult,
                            )
                            first = ci == 0 or chunk_tile[ci - 1] != t
                            last = (ci == t_total - 1
                                    or chunk_tile[ci + 1] != t)
                            if first and t % 4 == 0:
                                a_ps = pp.tile([128, 512], f32,
                                               space="PSUM", tag="a")
                            asl = slice((t % 4) * 128, (t % 4 + 1) * 128)
                            nc.tensor.matmul(
                                a_ps[:, asl],
                                m1n[:, (k - g0) * 128:(k - g0 + 1) * 128],
                                sel[:], start=first, stop=last)
                            if last and t % 4 == 3:
                                ast = wp.tile([128, 512], f32, tag="ast")
                                nc.vector.tensor_copy(ast[:], a_ps[:])
                                half = t // HALF
                                hc = (t - 3 - half * HALF) * 128
                                nc.sync.dma_start(
                                    a_h[half][:, hc:hc + 512], ast[:])
                            if last and t == HALF - 1:
                                nc.gpsimd.collective_compute(
                                    "AllReduce", mybir.AluOpType.add,
                                    replica_groups=[list(range(NCORES))],
                                    ins=[a_h[0].opt()], outs=[a_r[0].opt()])
                nc.gpsimd.collective_compute(
                    "AllReduce", mybir.AluOpType.add,
                    replica_groups=[list(range(NCORES))],
                    ins=[a_h[1].opt()], outs=[a_r[1].opt()])

                # ---- node phase (feature-major, replicated, fp32) ----
                ps = pp.tile([128, 128], f32, space="PSUM", tag="p1ps", bufs=2)
                nc.tensor.matmul(ps[:], w["w2t"][:], w["wu1_bot"][:],
                                 start=True, stop=True)
                w2up = wp.tile([128, 128], f32, tag="w2up")
                nc.vector.tensor_copy(w2up[:], ps[:])
                for j in range(NCH):
                    cols = slice(j * 512, (j + 1) * 512)
                    half = j // (NCH // 2)
                    hcols = slice((j - half * (NCH // 2)) * 512,
                                  (j - half * (NCH // 2) + 1) * 512)
                    at = sp.tile([128, 512], f32, tag="ain")
                    nc.sync.dma_start(at[:], a_r[half][:, hcols])
                    u1 = pp.tile([128, 512], f32, space="PSUM", tag="q")
                    nc.tensor.matmul(u1[:], w["wu1_top"][:], h_fm[:, cols],
                                     start=True, stop=False)
                    nc.tensor.matmul(u1[:], w2up[:], at[:],
                                     start=False, stop=not has_isolated)
                    if has_isolated:
                        nc.tensor.matmul(u1[:], w["c_row"][:],
                                         c_mask[:, cols],
                                         start=False, stop=True)
                    z1 = wp.tile([128, 512], f32, tag="z1")
                    nc.scalar.activation(z1[:], u1[:], Relu,
                                         bias=w["bu1_eff"][:])
                    u2 = pp.tile([128, 512], f32, space="PSUM", tag="a")
                    nc.tensor.matmul(u2[:], w["wu2"][:], z1[:],
                                     start=True, stop=True)
                    hn = wp.tile([128, 512], f32, tag="hn")
                    nc.vector.tensor_add(hn[:], h_fm[:, cols], u2[:])
                    nc.scalar.activation(h_fm[:, cols], hn[:], Relu,
                                         bias=w["bu2"][:])
                    if li == 0:
                        nc.vector.tensor_copy(h_bf[:, cols], h_fm[:, cols])

            # ---- prediction head (fp32) ----
            for j in range(NCH):
                cols = slice(j * 512, (j + 1) * 512)
                zp_ps = pp.tile([64, 512], f32, space="PSUM", tag="q")
                nc.tensor.matmul(zp_ps[:], c_wp1[:], h_fm[:, cols],
                                 start=True, stop=True)
                zp = wp.tile([64, 512], f32, tag="zp")
                nc.scalar.activation(zp[:], zp_ps[:], Relu, bias=c_bp1[:])
                y_ps = pp.tile([1, 512], f32, space="PSUM", tag="a")
                nc.tensor.matmul(y_ps[:], c_wp2[:], zp[:],
                                 start=True, stop=True)
                yt = wp.tile([1, 512], f32, tag="yt")
                nc.vector.tensor_scalar_add(yt[:], y_ps[:], c_bp2[:])
                nc.sync.dma_start(y_out[:, cols], yt[:])

    nc.compile()
    return nc


_CACHE = {}


def _get_program(meta_key, meta):
    if meta_key not in _CACHE:
        _CACHE[meta_key] = _build(meta)
    return _CACHE[meta_key]


def kernel(x, edge_index, edge_attr, edge_mark, deg_out, params,
           _trace=False):
    from concourse.bass_utils import run_bass_kernel_spmd

    in_maps, meta = _prep(x, edge_index, edge_attr, edge_mark, deg_out, params)
    meta_key = (meta["t_total"], tuple(meta["chunk_tile"]),
                meta["has_isolated"])
    nc = _get_program(meta_key, meta)

    kwargs = {}
    if _trace:
        # antenv.axon_hooks is absent in this image; provide the ctypes hook.
        try:
            import antenv.axon_hooks  # noqa: F401
        except ImportError:
            from trn_agent_boot.trn_boot import _ntff_profile_via_ctypes
            m = types.ModuleType("antenv.axon_hooks")
            m.get_axon_ntff_profile_hook = lambda: _ntff_profile_via_ctypes(
                "/opt/axon/libaxon_pjrt.so")
            sys.modules["antenv.axon_hooks"] = m
        kwargs["trace"] = True

    res = run_bass_kernel_spmd(nc, in_maps, list(range(NCORES)), **kwargs)
    y = np.asarray(res.results[0]["y"]).reshape(-1)[:N].astype(np.float32)
    if _trace:
        kernel._last_exec_time_ns = res.exec_time_ns
        kernel._last_results = res
    return y


# revision 13
# speedup vs baseline: 1.9622x; 1.2093x over previous
"""Trainium2 Bass kernel for 2-layer GNN message passing (CHARM).

Strategy (per the edge-sharding hint):
  - Edges are dealt round-robin across 8 NeuronCores from a global
    dst-sorted order, padded into 128-edge chunks aligned to 128-node
    destination tiles (same chunk structure on every core -> one SPMD
    program serves all 8).
  - msg1 is split: the node-dependent half (h @ W1a) is computed once per
    node as P1 (bf16) and gathered per edge via dma_gather; the
    edge-feature half ((edge_attr|edge_mark|1) @ (W1b|b1)) is a K=19 bf16
    matmul per chunk, accumulated with the gathered P1 in fp32 PSUM.
  - msg2 is linear, so it commutes with segment_sum:
    aggr = segment_sum(relu(m1)/deg) @ W2 (+ b2 folded into the update
    bias). The 1/deg is folded into the one-hot selection matrix.
  - segment_sum is a one-hot selection matmul accumulated in PSUM per node
    tile; partial aggregates A are AllReduce'd (fp32) in two halves so the
    first AllReduce overlaps the second half's edge compute.
  - The node-update MLP runs replicated (feature-major, fp32) per core.
"""

import math
import os
import sys
import types

import numpy as np

N = 10000
E = 320000
NP = 10240          # padded node count (80 tiles of 128)
NCORES = 8
HID = 128
TILE = 128
NT = NP // TILE     # 80 node tiles
ECORE = E // NCORES
GBLK = 8            # chunks per dma_gather (1024 idxs; >1024 crashes DGE ring)
NQ = 4              # SWDGE queues for gather descriptor generation
HALF = NT // 2      # node tiles per AllReduce half


def _prep(x, edge_index, edge_attr, edge_mark, deg_out, params):
    """Host-side sharding/layout prep. Returns (in_maps, meta)."""
    import ml_dtypes

    bf16 = ml_dtypes.bfloat16
    ei = np.asarray(edge_index)
    src_all = ei[0].astype(np.int64)
    dst_all = ei[1].astype(np.int64)
    ea = np.asarray(edge_attr, dtype=np.float32)
    em = np.asarray(edge_mark, dtype=np.float32)
    deg_out = np.asarray(deg_out, dtype=np.float32)
    x = np.asarray(x, dtype=np.float32)

    deg = np.where(deg_out == 0, 1.0, deg_out).astype(np.float32)
    invdeg = (1.0 / deg).astype(np.float32)
    has_isolated = bool((deg_out == 0).any())
    mask_pad = np.zeros((1, NP), np.float32)
    mask_pad[0, :N] = (deg_out > 0).astype(np.float32)

    # global dst-sort, then deal round-robin -> per-(core,tile) counts +-1
    gorder = np.argsort(dst_all, kind="stable")
    orders = [gorder[c::NCORES] for c in range(NCORES)]
    cnts = np.zeros((NCORES, NT), np.int64)
    for c in range(NCORES):
        cnts[c] = np.bincount(dst_all[orders[c]] // TILE, minlength=NT)
    nch = np.maximum(np.ceil(cnts / TILE).astype(np.int64).max(axis=0), 1)
    t_total = int(nch.sum())
    epad = t_total * TILE
    chunk_tile = np.repeat(np.arange(NT), nch)  # [t_total]

    ident = np.eye(128, dtype=np.float32)
    iota = np.tile(np.arange(128, dtype=np.float32)[None, :], (128, 1))
    x_fm = np.zeros((128, NP), np.float32)
    x_fm[:, :N] = x.T

    p = params
    invdeg_pad = np.ones(NP, np.float32)
    invdeg_pad[:N] = invdeg
    common = dict(
        x_fm=x_fm,
        ident=ident.astype(bf16),
        iota=iota,
        ivdn=np.ascontiguousarray(invdeg_pad.reshape(NT, 128).T),
        mask=mask_pad,
        win=np.asarray(p["in_proj"]["w"], np.float32),
        bin=np.asarray(p["in_proj"]["b"], np.float32).reshape(128, 1),
        wp1=np.asarray(p["pred1"]["w"], np.float32),
        bp1=np.asarray(p["pred1"]["b"], np.float32).reshape(64, 1),
        wp2=np.asarray(p["pred2"]["w"], np.float32),
        bp2=np.asarray(p["pred2"]["b"], np.float32).reshape(1, 1),
    )
    for li, lp in enumerate(p["layers"]):
        W1 = np.asarray(lp["msg1"]["w"], np.float32)
        b1 = np.asarray(lp["msg1"]["b"], np.float32)
        W2 = np.asarray(lp["msg2"]["w"], np.float32)
        b2 = np.asarray(lp["msg2"]["b"], np.float32)
        Wu1 = np.asarray(lp["up1"]["w"], np.float32)
        bu1 = np.asarray(lp["up1"]["b"], np.float32)
        Wu2 = np.asarray(lp["up2"]["w"], np.float32)
        bu2 = np.asarray(lp["up2"]["b"], np.float32)
        d = dict(
            w1a=W1[:HID].astype(bf16),
            w1b_aug=np.concatenate([W1[HID:], b1[None, :]], 0).astype(bf16),
            w2t=np.ascontiguousarray(W2.T),
            wu1_top=Wu1[:HID],
            wu1_bot=Wu1[HID:],
            bu1_eff=(bu1 + (0.0 if has_isolated else 1.0) * (b2 @ Wu1[HID:]))
            .astype(np.float32).reshape(128, 1),
            c_row=(b2 @ Wu1[HID:]).astype(np.float32).reshape(1, 128),
            wu2=Wu2,
            bu2=bu2.reshape(128, 1),
        )
        for k, v in d.items():
            common[f"l{li}_{k}"] = v

    in_maps = []
    for c in range(NCORES):
        o = orders[c]
        src = src_all[o]
        dst = dst_all[o]
        ecn = len(o)
        eaem = np.concatenate([ea[o], em[o], np.ones((ecn, 1), np.float32)], 1)
        src_p = np.zeros(epad, np.int64)
        eaem_p = np.zeros((epad, 19), np.float32)
        dstrel = np.full(epad, -1.0, np.float32)
        pos = 0
        epos = 0
        for t in range(NT):
            cnt = int(cnts[c, t])
            seg = slice(epos, epos + cnt)
            src_p[pos:pos + cnt] = src[seg]
            eaem_p[pos:pos + cnt] = eaem[seg]
            dstrel[pos:pos + cnt] = dst[seg] - t * TILE
            epos += cnt
            pos += int(nch[t]) * TILE
        # dma_gather idx layout: idx j -> partition j%16, col j//16, x8
        gidx = np.tile(src_p.astype(np.int16).reshape(epad // 16, 16).T, (8, 1))
        m = dict(common)
        m["gidx"] = np.ascontiguousarray(gidx)
        m["eaemT"] = np.ascontiguousarray(eaem_p.T).astype(bf16)  # [19, epad]
        m["dstrel"] = np.ascontiguousarray(
            dstrel.reshape(t_total, 128).T)                       # [128, T]
        in_maps.append(m)

    meta = dict(t_total=t_total, epad=epad, chunk_tile=chunk_tile.tolist(),
                has_isolated=has_isolated)
    return in_maps, meta


def _build(meta):
    import concourse.bacc as bacc
    import concourse.tile as tile
    from concourse import library_config, mybir
    from concourse._compat import get_trn_type

    f32 = mybir.dt.float32
    b16 = mybir.dt.bfloat16
    i16 = mybir.dt.int16
    Relu = mybir.ActivationFunctionType.Relu

    t_total = meta["t_total"]
    epad = meta["epad"]
    chunk_tile = meta["chunk_tile"]
    has_isolated = meta["has_isolated"]

    nc = bacc.Bacc(get_trn_type() or "TRN2", num_swdge_queues=NQ)

    def din(name, shape, dt=f32):
        return nc.dram_tensor(name, shape, dt, kind="ExternalInput")

    x_fm = din("x_fm", [128, NP])
    gidx = din("gidx", [128, epad // 16], i16)
    eaemT = din("eaemT", [19, epad], b16)
    dstrel = din("dstrel", [128, t_total])
    ivdn = din("ivdn", [128, NT])
    ident = din("ident", [128, 128], b16)
    iota = din("iota", [128, 128])
    mask = din("mask", [1, NP])
    win = din("win", [128, 128])
    bin_ = din("bin", [128, 1])
    wp1 = din("wp1", [128, 64])
    bp1 = din("bp1", [64, 1])
    wp2 = din("wp2", [64, 1])
    bp2 = din("bp2", [1, 1])
    L = []
    for li in range(2):
        L.append({k: din(f"l{li}_{k}", shp, dt) for k, shp, dt in [
            ("w1a", [128, 128], b16), ("w1b_aug", [19, 128], b16),
            ("w2t", [128, 128], f32),
            ("wu1_top", [128, 128], f32), ("wu1_bot", [128, 128], f32),
            ("bu1_eff", [128, 1], f32), ("c_row", [1, 128], f32),
            ("wu2", [128, 128], f32), ("bu2", [128, 1], f32)]})
    y_out = nc.dram_tensor("y", [1, NP], f32, kind="ExternalOutput")

    NCH = NP // 512  # 20 node-phase column chunks

    with tile.TileContext(nc) as tc:
        nc.gpsimd.load_library(library_config.mlp)
        with (
            tc.tile_pool(name="const", bufs=1) as cp,
            tc.tile_pool(name="stream", bufs=3) as sp,
            tc.tile_pool(name="work", bufs=3) as wp,
            tc.tile_pool(name="pgp", bufs=3) as pgp,
            tc.tile_pool(name="ep", bufs=3) as ep,
            tc.tile_pool(name="big", bufs=1) as bp,
            tc.tile_pool(name="psum", bufs=3, space="PSUM") as pp,
            tc.tile_pool(name="dram", bufs=2, space="DRAM") as dp,
        ):
            def load_const(ap, tag, dt=f32):
                t = cp.tile(ap.shape, dt, tag=tag)
                nc.sync.dma_start(t[:], ap[:])
                return t

            c_gidx = load_const(gidx, "c_gidx", i16)
            c_dstrel = load_const(dstrel, "c_dstrel")
            c_ivdn = load_const(ivdn, "c_ivdn")
            c_ident = load_const(ident, "c_ident", b16)
            c_iota = load_const(iota, "c_iota")
            c_mask = load_const(mask, "c_mask") if has_isolated else None
            c_win = load_const(win, "c_win")
            c_bin = load_const(bin_, "c_bin")
            c_wp1 = load_const(wp1, "c_wp1")
            c_bp1 = load_const(bp1, "c_bp1")
            c_wp2 = load_const(wp2, "c_wp2")
            c_bp2 = load_const(bp2, "c_bp2")
            cl = []
            for li in range(2):
                cl.append({
                    k: load_const(L[li][k], f"c_l{li}_{k}", L[li][k].dtype)
                    for k in L[li]})

            h_fm = bp.tile([128, NP], f32)      # resident features (fp32)
            h_bf = bp.tile([128, NP], b16)      # bf16 shadow for matmul lhsT

            # ---- in_proj: h = relu(Win.T @ x + bin) ----
            for j in range(NCH):
                cols = slice(j * 512, (j + 1) * 512)
                xt = sp.tile([128, 512], f32, tag="xin")
                nc.sync.dma_start(xt[:], x_fm[:, cols])
                ps = pp.tile([128, 512], f32, space="PSUM", tag="q")
                nc.tensor.matmul(ps[:], c_win[:], xt[:], start=True, stop=True)
                nc.scalar.activation(h_fm[:, cols], ps[:], Relu, bias=c_bin[:])
                nc.vector.tensor_copy(h_bf[:, cols], h_fm[:, cols])

            for li in range(2):
                w = cl[li]
                # ---- P1 = (h_bf.T @ W1a) in bf16, node-major in DRAM ----
                p1 = dp.tile([NP, 128], b16, tag="p1")
                for t in range(NT):
                    ncols = slice(t * 128, (t + 1) * 128)
                    ps = pp.tile([128, 128], f32, space="PSUM", tag="p1ps", bufs=2)
                    nc.tensor.matmul(ps[:], h_bf[:, ncols], w["w1a"][:],
                                     start=True, stop=True)
                    st = wp.tile([128, 128], b16, tag="p1st")
                    nc.vector.tensor_copy(st[:], ps[:])
                    nc.sync.dma_start(p1[ncols, :], st[:])

                # ---- edge phase ----
                a_h = [dp.tile([HALF * 128, 128], b16, tag=f"a_in{h}",
                               name=f"a_in{h}_l{li}")
                       for h in range(2)]
                a_r = [dp.tile([HALF * 128, 128], b16, tag=f"a_out{h}",
                               name=f"a_out{h}_l{li}")
                       for h in range(2)]
                nblocks = math.ceil(t_total / GBLK)
                a_ps = None
                for b in range(nblocks):
                    c0 = b * GBLK
                    bch = min(GBLK, t_total - c0)
                    pg = pgp.tile([128, bch, 128], b16, tag="pg")
                    nc.gpsimd.dma_gather(
                        pg[:], p1[:],
                        c_gidx[:, c0 * 8:(c0 + bch) * 8],
                        bch * 128, bch * 128, 128,
                        queue_num=b % NQ,
                    )
                    et = ep.tile([19, bch * 128], b16, tag="eaem")
                    nc.sync.dma_start(et[:],
                                      eaemT[:, c0 * 128:(c0 + bch) * 128])
                    for g0 in range(0, bch, 4):
                        gsz = min(4, bch - g0)
                        q = pp.tile([128, 512], f32, space="PSUM", tag="q")
                        for k in range(g0, g0 + gsz):
                            qs = slice((k - g0) * 128, (k - g0 + 1) * 128)
                            nc.tensor.matmul(
                                q[:, qs], et[:, k * 128:(k + 1) * 128],
                                w["w1b_aug"][:], start=True, stop=False)
                            nc.tensor.matmul(
                                q[:, qs], c_ident[:], pg[:, k, :],
                                start=False, stop=True)
                        m1n = wp.tile([128, 512], b16, tag="m1n")
                        nc.scalar.activation(m1n[:, :gsz * 128],
                                             q[:, :gsz * 128], Relu)
                        for k in range(g0, g0 + gsz):
                            ci = c0 + k
                            t = chunk_tile[ci]
                            sel = wp.tile([128, 128], b16, tag="sel")
                            nc.vector.tensor_tensor(
                                out=sel[:],
                                in0=c_dstrel[:, ci:ci + 1]
                                .to_broadcast([128, 128]),
                                in1=c_iota[:],
                                op=mybir.AluOpType.is_equal,
                            )
                            first = ci == 0 or chunk_tile[ci - 1] != t
                            last = (ci == t_total - 1
                                    or chunk_tile[ci + 1] != t)
                            if first and t % 4 == 0:
                                a_ps = pp.tile([128, 512], f32,
                                               space="PSUM", tag="a")
                            asl = slice((t % 4) * 128, (t % 4 + 1) * 128)
                            nc.tensor.matmul(
                                a_ps[:, asl], sel[:],
                                m1n[:, (k - g0) * 128:(k - g0 + 1) * 128],
                                start=first, stop=last)
                            if last:
                                ast = wp.tile([128, 128], b16, tag="ast")
                                nc.vector.tensor_scalar_mul(
                                    ast[:], a_ps[:, asl],
                                    c_ivdn[:, t:t + 1])
                                half = t // HALF
                                hr = (t - half * HALF) * 128
                                nc.sync.dma_start(
                                    a_h[half][hr:hr + 128, :], ast[:])
                            if last and t == HALF - 1:
                                nc.gpsimd.collective_compute(
                                    "AllReduce", mybir.AluOpType.add,
                                    replica_groups=[list(range(NCORES))],
                                    ins=[a_h[0].opt()], outs=[a_r[0].opt()])
                nc.gpsimd.collective_compute(
                    "AllReduce", mybir.AluOpType.add,
                    replica_groups=[list(range(NCORES))],
                    ins=[a_h[1].opt()], outs=[a_r[1].opt()])

                # ---- node phase (feature-major, replicated, fp32) ----
                ps = pp.tile([128, 128], f32, space="PSUM", tag="p1ps", bufs=2)
                nc.tensor.matmul(ps[:], w["w2t"][:], w["wu1_bot"][:],
                                 start=True, stop=True)
                w2up = wp.tile([128, 128], b16, tag="w2up")
                nc.vector.tensor_copy(w2up[:], ps[:])
                for j in range(NCH):
                    cols = slice(j * 512, (j + 1) * 512)
                    half = j // (NCH // 2)
                    hcols = slice((j - half * (NCH // 2)) * 512,
                                  (j - half * (NCH // 2) + 1) * 512)
                    at = sp.tile([128, 512], b16, tag="ain")
                    nc.sync.dma_start(
                        at[:],
                        a_r[half][hcols, :],
                        transpose=True)
                    u1 = pp.tile([128, 512], f32, space="PSUM", tag="q")
                    nc.tensor.matmul(u1[:], w["wu1_top"][:], h_fm[:, cols],
                                     start=True, stop=False)
                    nc.tensor.matmul(u1[:], w2up[:], at[:],
                                     start=False, stop=not has_isolated)
                    if has_isolated:
                        nc.tensor.matmul(u1[:], w["c_row"][:],
                                         c_mask[:, cols],
                                         start=False, stop=True)
                    z1 = wp.tile([128, 512], f32, tag="z1")
                    nc.scalar.activation(z1[:], u1[:], Relu,
                                         bias=w["bu1_eff"][:])
                    u2 = pp.tile([128, 512], f32, space="PSUM", tag="a")
                    nc.tensor.matmul(u2[:], w["wu2"][:], z1[:],
                                     start=True, stop=True)
                    hn = wp.tile([128, 512], f32, tag="hn")
                    nc.vector.tensor_add(hn[:], h_fm[:, cols], u2[:])
                    nc.scalar.activation(h_fm[:, cols], hn[:], Relu,
                                         bias=w["bu2"][:])
                    if li == 0:
                        nc.vector.tensor_copy(h_bf[:, cols], h_fm[:, cols])

            # ---- prediction head (fp32) ----
            for j in range(NCH):
                cols = slice(j * 512, (j + 1) * 512)
                zp_ps = pp.tile([64, 512], f32, space="PSUM", tag="q")
                nc.tensor.matmul(zp_ps[:], c_wp1[:], h_fm[:, cols],
                                 start=True, stop=True)
                zp = wp.tile([64, 512], f32, tag="zp")
                nc.scalar.activation(zp[:], zp_ps[:], Relu, bias=c_bp1[:])
                y_ps = pp.tile([1, 512], f32, space="PSUM", tag="a")
                nc.tensor.matmul(y_ps[:], c_wp2[:], zp[:],
                                 start=True, stop=True)
                yt = wp.tile([1, 512], f32, tag="yt")
                nc.vector.tensor_scalar_add(yt[:], y_ps[:], c_bp2[:])
                nc.sync.dma_start(y_out[:, cols], yt[:])

    nc.compile()
    return nc


_CACHE = {}


def _get_program(meta_key, meta):
    if meta_key not in _CACHE:
        _CACHE[meta_key] = _build(meta)
    return _CACHE[meta_key]


def kernel(x, edge_index, edge_attr, edge_mark, deg_out, params,
           _trace=False):
    from concourse.bass_utils import run_bass_kernel_spmd

    in_maps, meta = _prep(x, edge_index, edge_attr, edge_mark, deg_out, params)
    meta_key = (meta["t_total"], tuple(meta["chunk_tile"]),
                meta["has_isolated"])
    nc = _get_program(meta_key, meta)

    kwargs = {}
    if _trace:
        # antenv.axon_hooks is absent in this image; provide the ctypes hook.
        try:
            import antenv.axon_hooks  # noqa: F401
        except ImportError:
            from trn_agent_boot.trn_boot import _ntff_profile_via_ctypes
            m = types.ModuleType("antenv.axon_hooks")
            m.get_axon_ntff_profile_hook = lambda: _ntff_profile_via_ctypes(
                "/opt/axon/libaxon_pjrt.so")
            sys.modules["antenv.axon_hooks"] = m
        kwargs["trace"] = True

    res = run_bass_kernel_spmd(nc, in_maps, list(range(NCORES)), **kwargs)
    y = np.asarray(res.results[0]["y"]).reshape(-1)[:N].astype(np.float32)
    if _trace:
        kernel._last_exec_time_ns = res.exec_time_ns
        kernel._last_results = res
    return y


# revision 14
# speedup vs baseline: 2.2494x; 1.1464x over previous
"""Trainium2 Bass kernel for 2-layer GNN message passing (CHARM).

Strategy (per the edge-sharding hint):
  - Edges are dealt round-robin across 8 NeuronCores from a global
    dst-sorted order, padded into 128-edge chunks aligned to 128-node
    destination tiles (same chunk structure on every core -> one SPMD
    program serves all 8).
  - msg1 is split: the node-dependent half (h @ W1a) is computed once per
    node as P1 (bf16) and gathered per edge via dma_gather; the
    edge-feature half ((edge_attr|edge_mark|1) @ (W1b|b1)) is a K=19 bf16
    matmul per chunk, accumulated with the gathered P1 in fp32 PSUM.
  - msg2 is linear, so it commutes with segment_sum:
    aggr = segment_sum(relu(m1)/deg) @ W2 (+ b2 folded into the update
    bias). The 1/deg is folded into the one-hot selection matrix.
  - segment_sum is a one-hot selection matmul accumulated in PSUM per node
    tile; partial aggregates A are AllReduce'd (fp32) in two halves so the
    first AllReduce overlaps the second half's edge compute.
  - The node-update MLP runs replicated (feature-major, fp32) per core.
"""

import math
import os
import sys
import types

import numpy as np

N = 10000
E = 320000
NP = 10240          # padded node count (80 tiles of 128)
NCORES = 8
HID = 128
TILE = 128
NT = NP // TILE     # 80 node tiles
ECORE = E // NCORES
GBLK = 8            # chunks per dma_gather (1024 idxs; >1024 crashes DGE ring)
NQ = 4              # SWDGE queues for gather descriptor generation
HALF = NT // 2      # node tiles per AllReduce half


def _prep(x, edge_index, edge_attr, edge_mark, deg_out, params):
    """Host-side sharding/layout prep. Returns (in_maps, meta)."""
    import ml_dtypes

    bf16 = ml_dtypes.bfloat16
    ei = np.asarray(edge_index)
    src_all = ei[0].astype(np.int64)
    dst_all = ei[1].astype(np.int64)
    ea = np.asarray(edge_attr, dtype=np.float32)
    em = np.asarray(edge_mark, dtype=np.float32)
    deg_out = np.asarray(deg_out, dtype=np.float32)
    x = np.asarray(x, dtype=np.float32)

    deg = np.where(deg_out == 0, 1.0, deg_out).astype(np.float32)
    invdeg = (1.0 / deg).astype(np.float32)
    has_isolated = bool((deg_out == 0).any())
    mask_pad = np.zeros((1, NP), np.float32)
    mask_pad[0, :N] = (deg_out > 0).astype(np.float32)

    # global dst-sort, then deal round-robin -> per-(core,tile) counts +-1
    gorder = np.argsort(dst_all, kind="stable")
    orders = [gorder[c::NCORES] for c in range(NCORES)]
    cnts = np.zeros((NCORES, NT), np.int64)
    for c in range(NCORES):
        cnts[c] = np.bincount(dst_all[orders[c]] // TILE, minlength=NT)
    nch = np.maximum(np.ceil(cnts / TILE).astype(np.int64).max(axis=0), 1)
    t_total = int(nch.sum())
    epad = t_total * TILE
    chunk_tile = np.repeat(np.arange(NT), nch)  # [t_total]

    ident = np.eye(128, dtype=np.float32)
    iota = np.tile(np.arange(128, dtype=np.float32)[None, :], (128, 1))
    x_fm = np.zeros((128, NP), np.float32)
    x_fm[:, :N] = x.T

    p = params
    invdeg_pad = np.ones(NP, np.float32)
    invdeg_pad[:N] = invdeg
    common = dict(
        x_fm=x_fm,
        ident=ident.astype(bf16),
        iota=iota,
        ivdn=np.ascontiguousarray(invdeg_pad.reshape(NT, 128).T),
        mask=mask_pad,
        win=np.asarray(p["in_proj"]["w"], np.float32),
        bin=np.asarray(p["in_proj"]["b"], np.float32).reshape(128, 1),
        wp1=np.asarray(p["pred1"]["w"], np.float32),
        bp1=np.asarray(p["pred1"]["b"], np.float32).reshape(64, 1),
        wp2=np.asarray(p["pred2"]["w"], np.float32),
        bp2=np.asarray(p["pred2"]["b"], np.float32).reshape(1, 1),
    )
    for li, lp in enumerate(p["layers"]):
        W1 = np.asarray(lp["msg1"]["w"], np.float32)
        b1 = np.asarray(lp["msg1"]["b"], np.float32)
        W2 = np.asarray(lp["msg2"]["w"], np.float32)
        b2 = np.asarray(lp["msg2"]["b"], np.float32)
        Wu1 = np.asarray(lp["up1"]["w"], np.float32)
        bu1 = np.asarray(lp["up1"]["b"], np.float32)
        Wu2 = np.asarray(lp["up2"]["w"], np.float32)
        bu2 = np.asarray(lp["up2"]["b"], np.float32)
        d = dict(
            w1a=W1[:HID].astype(bf16),
            w1b_aug=np.concatenate([W1[HID:], b1[None, :]], 0).astype(bf16),
            w2t=np.ascontiguousarray(W2.T),
            wu1_top=Wu1[:HID].astype(bf16),
            wu1_bot=Wu1[HID:],
            bu1_eff=(bu1 + (0.0 if has_isolated else 1.0) * (b2 @ Wu1[HID:]))
            .astype(np.float32).reshape(128, 1),
            c_row=(b2 @ Wu1[HID:]).astype(np.float32).reshape(1, 128),
            wu2=Wu2.astype(bf16),
            bu2=bu2.reshape(128, 1),
        )
        for k, v in d.items():
            common[f"l{li}_{k}"] = v

    in_maps = []
    for c in range(NCORES):
        o = orders[c]
        src = src_all[o]
        dst = dst_all[o]
        ecn = len(o)
        eaem = np.concatenate([ea[o], em[o], np.ones((ecn, 1), np.float32)], 1)
        src_p = np.zeros(epad, np.int64)
        eaem_p = np.zeros((epad, 19), np.float32)
        dstrel = np.full(epad, -1.0, np.float32)
        pos = 0
        epos = 0
        for t in range(NT):
            cnt = int(cnts[c, t])
            seg = slice(epos, epos + cnt)
            src_p[pos:pos + cnt] = src[seg]
            eaem_p[pos:pos + cnt] = eaem[seg]
            dstrel[pos:pos + cnt] = dst[seg] - t * TILE
            epos += cnt
            pos += int(nch[t]) * TILE
        # dma_gather idx layout: idx j -> partition j%16, col j//16, x8
        gidx = np.tile(src_p.astype(np.int16).reshape(epad // 16, 16).T, (8, 1))
        m = dict(common)
        m["gidx"] = np.ascontiguousarray(gidx)
        m["eaemT"] = np.ascontiguousarray(eaem_p.T).astype(bf16)  # [19, epad]
        m["dstrel"] = np.ascontiguousarray(
            dstrel.reshape(t_total, 128).T)                       # [128, T]
        in_maps.append(m)

    meta = dict(t_total=t_total, epad=epad, chunk_tile=chunk_tile.tolist(),
                has_isolated=has_isolated)
    return in_maps, meta


def _build(meta):
    import concourse.bacc as bacc
    import concourse.tile as tile
    from concourse import library_config, mybir
    from concourse._compat import get_trn_type

    f32 = mybir.dt.float32
    b16 = mybir.dt.bfloat16
    i16 = mybir.dt.int16
    Relu = mybir.ActivationFunctionType.Relu

    t_total = meta["t_total"]
    epad = meta["epad"]
    chunk_tile = meta["chunk_tile"]
    has_isolated = meta["has_isolated"]

    nc = bacc.Bacc(get_trn_type() or "TRN2", num_swdge_queues=NQ)

    def din(name, shape, dt=f32):
        return nc.dram_tensor(name, shape, dt, kind="ExternalInput")

    x_fm = din("x_fm", [128, NP])
    gidx = din("gidx", [128, epad // 16], i16)
    eaemT = din("eaemT", [19, epad], b16)
    dstrel = din("dstrel", [128, t_total])
    ivdn = din("ivdn", [128, NT])
    ident = din("ident", [128, 128], b16)
    iota = din("iota", [128, 128])
    mask = din("mask", [1, NP])
    win = din("win", [128, 128])
    bin_ = din("bin", [128, 1])
    wp1 = din("wp1", [128, 64])
    bp1 = din("bp1", [64, 1])
    wp2 = din("wp2", [64, 1])
    bp2 = din("bp2", [1, 1])
    L = []
    for li in range(2):
        L.append({k: din(f"l{li}_{k}", shp, dt) for k, shp, dt in [
            ("w1a", [128, 128], b16), ("w1b_aug", [19, 128], b16),
            ("w2t", [128, 128], f32),
            ("wu1_top", [128, 128], b16), ("wu1_bot", [128, 128], f32),
            ("bu1_eff", [128, 1], f32), ("c_row", [1, 128], f32),
            ("wu2", [128, 128], b16), ("bu2", [128, 1], f32)]})
    y_out = nc.dram_tensor("y", [1, NP], f32, kind="ExternalOutput")

    NCH = NP // 512  # 20 node-phase column chunks

    with tile.TileContext(nc) as tc:
        nc.gpsimd.load_library(library_config.mlp)
        with (
            tc.tile_pool(name="const", bufs=1) as cp,
            tc.tile_pool(name="stream", bufs=3) as sp,
            tc.tile_pool(name="work", bufs=3) as wp,
            tc.tile_pool(name="pgp", bufs=3) as pgp,
            tc.tile_pool(name="ep", bufs=3) as ep,
            tc.tile_pool(name="big", bufs=1) as bp,
            tc.tile_pool(name="psum", bufs=3, space="PSUM") as pp,
            tc.tile_pool(name="dram", bufs=2, space="DRAM") as dp,
        ):
            def load_const(ap, tag, dt=f32):
                t = cp.tile(ap.shape, dt, tag=tag)
                nc.sync.dma_start(t[:], ap[:])
                return t

            c_gidx = load_const(gidx, "c_gidx", i16)
            c_dstrel = load_const(dstrel, "c_dstrel")
            c_ivdn = load_const(ivdn, "c_ivdn")
            c_ident = load_const(ident, "c_ident", b16)
            c_iota = load_const(iota, "c_iota")
            c_mask = load_const(mask, "c_mask") if has_isolated else None
            c_win = load_const(win, "c_win")
            c_bin = load_const(bin_, "c_bin")
            c_wp1 = load_const(wp1, "c_wp1")
            c_bp1 = load_const(bp1, "c_bp1")
            c_wp2 = load_const(wp2, "c_wp2")
            c_bp2 = load_const(bp2, "c_bp2")
            cl = []
            for li in range(2):
                cl.append({
                    k: load_const(L[li][k], f"c_l{li}_{k}", L[li][k].dtype)
                    for k in L[li]})

            h_fm = bp.tile([128, NP], f32)      # resident features (fp32)
            h_bf = bp.tile([128, NP], b16)      # bf16 shadow for matmul lhsT

            # ---- in_proj: h = relu(Win.T @ x + bin) ----
            for j in range(NCH):
                cols = slice(j * 512, (j + 1) * 512)
                xt = sp.tile([128, 512], f32, tag="xin")
                nc.sync.dma_start(xt[:], x_fm[:, cols])
                ps = pp.tile([128, 512], f32, space="PSUM", tag="q")
                nc.tensor.matmul(ps[:], c_win[:], xt[:], start=True, stop=True)
                nc.scalar.activation(h_fm[:, cols], ps[:], Relu, bias=c_bin[:])
                nc.vector.tensor_copy(h_bf[:, cols], h_fm[:, cols])

            for li in range(2):
                w = cl[li]
                # ---- P1 = (h_bf.T @ W1a) in bf16, node-major in DRAM ----
                p1 = dp.tile([NP, 128], b16, tag="p1")
                for t0 in range(0, NT, 4):
                    ps = pp.tile([128, 512], f32, space="PSUM", tag="p1ps",
                                 bufs=2)
                    for t in range(t0, t0 + 4):
                        qs = slice((t - t0) * 128, (t - t0 + 1) * 128)
                        nc.tensor.matmul(
                            ps[:, qs], h_bf[:, t * 128:(t + 1) * 128],
                            w["w1a"][:], start=True, stop=True)
                    st = wp.tile([128, 4, 128], b16, tag="p1st")
                    nc.vector.tensor_copy(st[:], ps[:])
                    nc.sync.dma_start(
                        p1[t0 * 128:(t0 + 4) * 128, :].rearrange(
                            "(k p) f -> p k f", p=128), st[:])

                # ---- edge phase ----
                a_h = [dp.tile([HALF * 128, 128], b16, tag=f"a_in{h}",
                               name=f"a_in{h}_l{li}")
                       for h in range(2)]
                a_r = [dp.tile([HALF * 128, 128], b16, tag=f"a_out{h}",
                               name=f"a_out{h}_l{li}")
                       for h in range(2)]
                nblocks = math.ceil(t_total / GBLK)
                a_ps = None
                for b in range(nblocks):
                    c0 = b * GBLK
                    bch = min(GBLK, t_total - c0)
                    pg = pgp.tile([128, bch, 128], b16, tag="pg")
                    nc.gpsimd.dma_gather(
                        pg[:], p1[:],
                        c_gidx[:, c0 * 8:(c0 + bch) * 8],
                        bch * 128, bch * 128, 128,
                        queue_num=b % NQ,
                    )
                    et = ep.tile([19, bch * 128], b16, tag="eaem")
                    nc.sync.dma_start(et[:],
                                      eaemT[:, c0 * 128:(c0 + bch) * 128])
                    for g0 in range(0, bch, 4):
                        gsz = min(4, bch - g0)
                        q = pp.tile([128, 512], f32, space="PSUM", tag="q")
                        nc.tensor.matmul(
                            q[:, :gsz * 128], c_ident[:],
                            pg[:, g0:g0 + gsz, :], start=True, stop=False)
                        for k in range(g0, g0 + gsz):
                            qs = slice((k - g0) * 128, (k - g0 + 1) * 128)
                            nc.tensor.matmul(
                                q[:, qs], et[:, k * 128:(k + 1) * 128],
                                w["w1b_aug"][:], start=False, stop=True)
                        m1n = wp.tile([128, 512], b16, tag="m1n")
                        nc.scalar.activation(m1n[:, :gsz * 128],
                                             q[:, :gsz * 128], Relu)
                        for k in range(g0, g0 + gsz):
                            ci = c0 + k
                            t = chunk_tile[ci]
                            sel = wp.tile([128, 128], b16, tag="sel")
                            nc.vector.tensor_tensor(
                                out=sel[:],
                                in0=c_dstrel[:, ci:ci + 1]
                                .to_broadcast([128, 128]),
                                in1=c_iota[:],
                                op=mybir.AluOpType.is_equal,
                            )
                            first = ci == 0 or chunk_tile[ci - 1] != t
                            last = (ci == t_total - 1
                                    or chunk_tile[ci + 1] != t)
                            if first and t % 4 == 0:
                                a_ps = pp.tile([128, 512], f32,
                                               space="PSUM", tag="a")
                            asl = slice((t % 4) * 128, (t % 4 + 1) * 128)
                            nc.tensor.matmul(
                                a_ps[:, asl], sel[:],
                                m1n[:, (k - g0) * 128:(k - g0 + 1) * 128],
                                start=first, stop=last)
                            if last:
                                ast = wp.tile([128, 128], b16, tag="ast")
                                nc.vector.tensor_scalar_mul(
                                    ast[:], a_ps[:, asl],
                                    c_ivdn[:, t:t + 1])
                                half = t // HALF
                                hr = (t - half * HALF) * 128
                                nc.sync.dma_start(
                                    a_h[half][hr:hr + 128, :], ast[:])
                            if last and t == HALF - 1:
                                nc.gpsimd.collective_compute(
                                    "AllReduce", mybir.AluOpType.add,
                                    replica_groups=[list(range(NCORES))],
                                    ins=[a_h[0].opt()], outs=[a_r[0].opt()])
                nc.gpsimd.collective_compute(
                    "AllReduce", mybir.AluOpType.add,
                    replica_groups=[list(range(NCORES))],
                    ins=[a_h[1].opt()], outs=[a_r[1].opt()])

                # ---- node phase (feature-major, replicated, fp32) ----
                ps = pp.tile([128, 128], f32, space="PSUM", tag="p1ps", bufs=2)
                nc.tensor.matmul(ps[:], w["w2t"][:], w["wu1_bot"][:],
                                 start=True, stop=True)
                w2up = wp.tile([128, 128], b16, tag="w2up")
                nc.vector.tensor_copy(w2up[:], ps[:])
                for j in range(NCH):
                    cols = slice(j * 512, (j + 1) * 512)
                    half = j // (NCH // 2)
                    hcols = slice((j - half * (NCH // 2)) * 512,
                                  (j - half * (NCH // 2) + 1) * 512)
                    at = sp.tile([128, 512], b16, tag="ain")
                    nc.sync.dma_start(
                        at[:],
                        a_r[half][hcols, :],
                        transpose=True)
                    u1 = pp.tile([128, 512], f32, space="PSUM", tag="q")
                    nc.tensor.matmul(u1[:], w["wu1_top"][:], h_bf[:, cols],
                                     start=True, stop=False)
                    nc.tensor.matmul(u1[:], w2up[:], at[:],
                                     start=False, stop=not has_isolated)
                    if has_isolated:
                        nc.tensor.matmul(u1[:], w["c_row"][:],
                                         c_mask[:, cols],
                                         start=False, stop=True)
                    z1 = wp.tile([128, 512], b16, tag="z1")
                    nc.scalar.activation(z1[:], u1[:], Relu,
                                         bias=w["bu1_eff"][:])
                    u2 = pp.tile([128, 512], f32, space="PSUM", tag="a")
                    nc.tensor.matmul(u2[:], w["wu2"][:], z1[:],
                                     start=True, stop=True)
                    hn = wp.tile([128, 512], f32, tag="hn")
                    nc.vector.tensor_add(hn[:], h_fm[:, cols], u2[:])
                    nc.scalar.activation(h_fm[:, cols], hn[:], Relu,
                                         bias=w["bu2"][:])
                    if li == 0:
                        nc.vector.tensor_copy(h_bf[:, cols], h_fm[:, cols])

            # ---- prediction head (fp32) ----
            for j in range(NCH):
                cols = slice(j * 512, (j + 1) * 512)
                zp_ps = pp.tile([64, 512], f32, space="PSUM", tag="q")
                nc.tensor.matmul(zp_ps[:], c_wp1[:], h_fm[:, cols],
                                 start=True, stop=True)
                zp = wp.tile([64, 512], f32, tag="zp")
                nc.scalar.activation(zp[:], zp_ps[:], Relu, bias=c_bp1[:])
                y_ps = pp.tile([1, 512], f32, space="PSUM", tag="a")
                nc.tensor.matmul(y_ps[:], c_wp2[:], zp[:],
                                 start=True, stop=True)
                yt = wp.tile([1, 512], f32, tag="yt")
                nc.vector.tensor_scalar_add(yt[:], y_ps[:], c_bp2[:])
                nc.sync.dma_start(y_out[:, cols], yt[:])

    nc.compile()
    return nc


_CACHE = {}


def _get_program(meta_key, meta):
    if meta_key not in _CACHE:
        _CACHE[meta_key] = _build(meta)
    return _CACHE[meta_key]


def kernel(x, edge_index, edge_attr, edge_mark, deg_out, params,
           _trace=False):
    from concourse.bass_utils import run_bass_kernel_spmd

    in_maps, meta = _prep(x, edge_index, edge_attr, edge_mark, deg_out, params)
    meta_key = (meta["t_total"], tuple(meta["chunk_tile"]),
                meta["has_isolated"])
    nc = _get_program(meta_key, meta)

    kwargs = {}
    if _trace:
        # antenv.axon_hooks is absent in this image; provide the ctypes hook.
        try:
            import antenv.axon_hooks  # noqa: F401
        except ImportError:
            from trn_agent_boot.trn_boot import _ntff_profile_via_ctypes
            m = types.ModuleType("antenv.axon_hooks")
            m.get_axon_ntff_profile_hook = lambda: _ntff_profile_via_ctypes(
                "/opt/axon/libaxon_pjrt.so")
            sys.modules["antenv.axon_hooks"] = m
        kwargs["trace"] = True

    res = run_bass_kernel_spmd(nc, in_maps, list(range(NCORES)), **kwargs)
    y = np.asarray(res.results[0]["y"]).reshape(-1)[:N].astype(np.float32)
    if _trace:
        kernel._last_exec_time_ns = res.exec_time_ns
        kernel._last_results = res
    return y


# revision 16
# speedup vs baseline: 2.4836x; 1.1041x over previous
"""Trainium2 Bass kernel for 2-layer GNN message passing (CHARM).

Strategy (per the edge-sharding hint):
  - Edges are dealt round-robin across 8 NeuronCores from a global
    dst-sorted order, padded into 128-edge chunks aligned to 128-node
    destination tiles (same chunk structure on every core -> one SPMD
    program serves all 8).
  - msg1 is split: the node-dependent half (h @ W1a) is computed once per
    node as P1 (bf16) and gathered per edge via dma_gather; the
    edge-feature half ((edge_attr|edge_mark|1) @ (W1b|b1)) is a K=19 bf16
    matmul per chunk, accumulated with the gathered P1 in fp32 PSUM.
  - msg2 is linear, so it commutes with segment_sum:
    aggr = segment_sum(relu(m1)/deg) @ W2 (+ b2 folded into the update
    bias). The 1/deg is folded into the one-hot selection matrix.
  - segment_sum is a one-hot selection matmul accumulated in PSUM per node
    tile; partial aggregates A are AllReduce'd (fp32) in two halves so the
    first AllReduce overlaps the second half's edge compute.
  - The node-update MLP runs replicated (feature-major, fp32) per core.
"""

import math
import os
import sys
import types

import numpy as np

N = 10000
E = 320000
NP = 10240          # padded node count (80 tiles of 128)
NCORES = 8
HID = 128
TILE = 128
NT = NP // TILE     # 80 node tiles
ECORE = E // NCORES
GBLK = 8            # chunks per dma_gather (1024 idxs; >1024 crashes DGE ring)
NQ = 4              # SWDGE queues for gather descriptor generation
HALF = NT // 2      # node tiles per AllReduce half


def _prep(x, edge_index, edge_attr, edge_mark, deg_out, params):
    """Host-side sharding/layout prep. Returns (in_maps, meta)."""
    import ml_dtypes

    bf16 = ml_dtypes.bfloat16
    ei = np.asarray(edge_index)
    src_all = ei[0].astype(np.int64)
    dst_all = ei[1].astype(np.int64)
    ea = np.asarray(edge_attr, dtype=np.float32)
    em = np.asarray(edge_mark, dtype=np.float32)
    deg_out = np.asarray(deg_out, dtype=np.float32)
    x = np.asarray(x, dtype=np.float32)

    deg = np.where(deg_out == 0, 1.0, deg_out).astype(np.float32)
    invdeg = (1.0 / deg).astype(np.float32)
    has_isolated = bool((deg_out == 0).any())
    mask_pad = np.zeros((1, NP), np.float32)
    mask_pad[0, :N] = (deg_out > 0).astype(np.float32)

    # global dst-sort, then deal round-robin -> per-(core,tile) counts +-1
    gorder = np.argsort(dst_all, kind="stable")
    orders = [gorder[c::NCORES] for c in range(NCORES)]
    cnts = np.zeros((NCORES, NT), np.int64)
    for c in range(NCORES):
        cnts[c] = np.bincount(dst_all[orders[c]] // TILE, minlength=NT)
    nch = np.maximum(np.ceil(cnts / TILE).astype(np.int64).max(axis=0), 1)
    t_total = int(nch.sum())
    epad = t_total * TILE
    chunk_tile = np.repeat(np.arange(NT), nch)  # [t_total]

    ident = np.eye(128, dtype=np.float32)
    iota = np.tile(np.arange(128, dtype=np.float32)[None, :], (128, 1))
    x_fm = np.zeros((128, NP), np.float32)
    x_fm[:, :N] = x.T

    p = params
    invdeg_pad = np.ones(NP, np.float32)
    invdeg_pad[:N] = invdeg
    common = dict(
        x_fm=x_fm,
        ident=ident.astype(bf16),
        iota=iota,
        ivdn=np.ascontiguousarray(invdeg_pad.reshape(NT, 128).T),
        mask=mask_pad,
        win=np.asarray(p["in_proj"]["w"], np.float32),
        bin=np.asarray(p["in_proj"]["b"], np.float32).reshape(128, 1),
        wp1=np.asarray(p["pred1"]["w"], np.float32),
        bp1=np.asarray(p["pred1"]["b"], np.float32).reshape(64, 1),
        wp2=np.asarray(p["pred2"]["w"], np.float32),
        bp2=np.asarray(p["pred2"]["b"], np.float32).reshape(1, 1),
    )
    for li, lp in enumerate(p["layers"]):
        W1 = np.asarray(lp["msg1"]["w"], np.float32)
        b1 = np.asarray(lp["msg1"]["b"], np.float32)
        W2 = np.asarray(lp["msg2"]["w"], np.float32)
        b2 = np.asarray(lp["msg2"]["b"], np.float32)
        Wu1 = np.asarray(lp["up1"]["w"], np.float32)
        bu1 = np.asarray(lp["up1"]["b"], np.float32)
        Wu2 = np.asarray(lp["up2"]["w"], np.float32)
        bu2 = np.asarray(lp["up2"]["b"], np.float32)
        d = dict(
            w1a=W1[:HID].astype(bf16),
            w1b_aug=np.concatenate([W1[HID:], b1[None, :]], 0).astype(bf16),
            w2t=np.ascontiguousarray(W2.T),
            wu1_top=Wu1[:HID].astype(bf16),
            wu1_bot=Wu1[HID:],
            bu1_eff=(bu1 + (0.0 if has_isolated else 1.0) * (b2 @ Wu1[HID:]))
            .astype(np.float32).reshape(128, 1),
            c_row=(b2 @ Wu1[HID:]).astype(np.float32).reshape(1, 128),
            wu2=Wu2.astype(bf16),
            bu2=bu2.reshape(128, 1),
        )
        for k, v in d.items():
            common[f"l{li}_{k}"] = v

    in_maps = []
    for c in range(NCORES):
        o = orders[c]
        src = src_all[o]
        dst = dst_all[o]
        ecn = len(o)
        eaem = np.concatenate([ea[o], em[o], np.ones((ecn, 1), np.float32)], 1)
        src_p = np.zeros(epad, np.int64)
        eaem_p = np.zeros((epad, 19), np.float32)
        dstrel = np.full(epad, -1.0, np.float32)
        pos = 0
        epos = 0
        for t in range(NT):
            cnt = int(cnts[c, t])
            seg = slice(epos, epos + cnt)
            src_p[pos:pos + cnt] = src[seg]
            eaem_p[pos:pos + cnt] = eaem[seg]
            dstrel[pos:pos + cnt] = dst[seg] - t * TILE
            epos += cnt
            pos += int(nch[t]) * TILE
        # dma_gather idx layout: idx j -> partition j%16, col j//16, x8
        gidx = np.tile(src_p.astype(np.int16).reshape(epad // 16, 16).T, (8, 1))
        m = dict(common)
        m["gidx"] = np.ascontiguousarray(gidx)
        m["eaemT"] = np.ascontiguousarray(eaem_p.T).astype(bf16)  # [19, epad]
        m["dstrel"] = np.ascontiguousarray(
            dstrel.reshape(t_total, 128).T)                       # [128, T]
        in_maps.append(m)

    meta = dict(t_total=t_total, epad=epad, chunk_tile=chunk_tile.tolist(),
                has_isolated=has_isolated)
    return in_maps, meta


def _build(meta):
    import concourse.bacc as bacc
    import concourse.tile as tile
    from concourse import library_config, mybir
    from concourse._compat import get_trn_type

    f32 = mybir.dt.float32
    b16 = mybir.dt.bfloat16
    i16 = mybir.dt.int16
    Relu = mybir.ActivationFunctionType.Relu

    t_total = meta["t_total"]
    epad = meta["epad"]
    chunk_tile = meta["chunk_tile"]
    has_isolated = meta["has_isolated"]

    nc = bacc.Bacc(get_trn_type() or "TRN2", num_swdge_queues=NQ)

    def din(name, shape, dt=f32):
        return nc.dram_tensor(name, shape, dt, kind="ExternalInput")

    x_fm = din("x_fm", [128, NP])
    gidx = din("gidx", [128, epad // 16], i16)
    eaemT = din("eaemT", [19, epad], b16)
    dstrel = din("dstrel", [128, t_total])
    ivdn = din("ivdn", [128, NT])
    ident = din("ident", [128, 128], b16)
    iota = din("iota", [128, 128])
    mask = din("mask", [1, NP])
    win = din("win", [128, 128])
    bin_ = din("bin", [128, 1])
    wp1 = din("wp1", [128, 64])
    bp1 = din("bp1", [64, 1])
    wp2 = din("wp2", [64, 1])
    bp2 = din("bp2", [1, 1])
    L = []
    for li in range(2):
        L.append({k: din(f"l{li}_{k}", shp, dt) for k, shp, dt in [
            ("w1a", [128, 128], b16), ("w1b_aug", [19, 128], b16),
            ("w2t", [128, 128], f32),
            ("wu1_top", [128, 128], b16), ("wu1_bot", [128, 128], f32),
            ("bu1_eff", [128, 1], f32), ("c_row", [1, 128], f32),
            ("wu2", [128, 128], b16), ("bu2", [128, 1], f32)]})
    y_out = nc.dram_tensor("y", [1, NP], f32, kind="ExternalOutput")

    NCH = NP // 512  # 20 node-phase column chunks

    with tile.TileContext(nc) as tc:
        nc.gpsimd.load_library(library_config.mlp)
        with (
            tc.tile_pool(name="const", bufs=1) as cp,
            tc.tile_pool(name="stream", bufs=3) as sp,
            tc.tile_pool(name="work", bufs=4) as wp,
            tc.tile_pool(name="pgp", bufs=4) as pgp,
            tc.tile_pool(name="ep", bufs=4) as ep,
            tc.tile_pool(name="big", bufs=1) as bp,
            tc.tile_pool(name="psum", bufs=3, space="PSUM") as pp,
            tc.tile_pool(name="dram", bufs=2, space="DRAM") as dp,
        ):
            def load_const(ap, tag, dt=f32):
                t = cp.tile(ap.shape, dt, tag=tag)
                nc.sync.dma_start(t[:], ap[:])
                return t

            c_gidx = load_const(gidx, "c_gidx", i16)
            c_dstrel = load_const(dstrel, "c_dstrel")
            c_ivdn = load_const(ivdn, "c_ivdn")
            c_ident = load_const(ident, "c_ident", b16)
            c_iota = load_const(iota, "c_iota")
            c_mask = load_const(mask, "c_mask") if has_isolated else None
            c_win = load_const(win, "c_win")
            c_bin = load_const(bin_, "c_bin")
            c_wp1 = load_const(wp1, "c_wp1")
            c_bp1 = load_const(bp1, "c_bp1")
            c_wp2 = load_const(wp2, "c_wp2")
            c_bp2 = load_const(bp2, "c_bp2")
            cl = []
            for li in range(2):
                cl.append({
                    k: load_const(L[li][k], f"c_l{li}_{k}", L[li][k].dtype)
                    for k in L[li]})

            h_fm = bp.tile([128, NP], f32)      # resident features (fp32)
            h_bf = bp.tile([128, NP], b16)      # bf16 shadow for matmul lhsT

            # ---- in_proj: h = relu(Win.T @ x + bin) ----
            for j in range(NCH):
                cols = slice(j * 512, (j + 1) * 512)
                xt = sp.tile([128, 512], f32, tag="xin")
                nc.sync.dma_start(xt[:], x_fm[:, cols])
                ps = pp.tile([128, 512], f32, space="PSUM", tag="q", bufs=4)
                nc.tensor.matmul(ps[:], c_win[:], xt[:], start=True, stop=True)
                nc.scalar.activation(h_fm[:, cols], ps[:], Relu, bias=c_bin[:])
                nc.vector.tensor_copy(h_bf[:, cols], h_fm[:, cols])

            for li in range(2):
                w = cl[li]
                # ---- P1 = (h_bf.T @ W1a) in bf16, node-major in DRAM ----
                p1 = dp.tile([NP, 128], b16, tag="p1")
                for t0 in range(0, NT, 4):
                    ps = pp.tile([128, 512], f32, space="PSUM", tag="p1ps",
                                 bufs=2)
                    for t in range(t0, t0 + 4):
                        qs = slice((t - t0) * 128, (t - t0 + 1) * 128)
                        nc.tensor.matmul(
                            ps[:, qs], h_bf[:, t * 128:(t + 1) * 128],
                            w["w1a"][:], start=True, stop=True)
                    st = wp.tile([128, 4, 128], b16, tag="p1st")
                    nc.vector.tensor_copy(st[:], ps[:])
                    nc.sync.dma_start(
                        p1[t0 * 128:(t0 + 4) * 128, :].rearrange(
                            "(k p) f -> p k f", p=128), st[:])

                # ---- edge phase ----
                a_h = [dp.tile([HALF * 128, 128], b16, tag=f"a_in{h}",
                               name=f"a_in{h}_l{li}")
                       for h in range(2)]
                a_r = [dp.tile([HALF * 128, 128], b16, tag=f"a_out{h}",
                               name=f"a_out{h}_l{li}")
                       for h in range(2)]
                nblocks = math.ceil(t_total / GBLK)
                a_ps = None
                for b in range(nblocks):
                    c0 = b * GBLK
                    bch = min(GBLK, t_total - c0)
                    pg = pgp.tile([128, bch, 128], b16, tag="pg")
                    nc.gpsimd.dma_gather(
                        pg[:], p1[:],
                        c_gidx[:, c0 * 8:(c0 + bch) * 8],
                        bch * 128, bch * 128, 128,
                        queue_num=b % NQ,
                    )
                    et = ep.tile([19, bch * 128], b16, tag="eaem")
                    nc.sync.dma_start(et[:],
                                      eaemT[:, c0 * 128:(c0 + bch) * 128])
                    for g0 in range(0, bch, 4):
                        gsz = min(4, bch - g0)
                        q = pp.tile([128, 512], f32, space="PSUM", tag="q", bufs=4)
                        nc.tensor.matmul(
                            q[:, :gsz * 128], c_ident[:],
                            pg[:, g0:g0 + gsz, :], start=True, stop=False)
                        for k in range(g0, g0 + gsz):
                            qs = slice((k - g0) * 128, (k - g0 + 1) * 128)
                            nc.tensor.matmul(
                                q[:, qs], et[:, k * 128:(k + 1) * 128],
                                w["w1b_aug"][:], start=False, stop=True)
                        m1n = wp.tile([128, 512], b16, tag="m1n")
                        nc.scalar.activation(m1n[:, :gsz * 128],
                                             q[:, :gsz * 128], Relu)
                        for k in range(g0, g0 + gsz):
                            ci = c0 + k
                            t = chunk_tile[ci]
                            sel = wp.tile([128, 128], b16, tag="sel")
                            nc.vector.tensor_tensor(
                                out=sel[:],
                                in0=c_dstrel[:, ci:ci + 1]
                                .to_broadcast([128, 128]),
                                in1=c_iota[:],
                                op=mybir.AluOpType.is_equal,
                            )
                            first = ci == 0 or chunk_tile[ci - 1] != t
                            last = (ci == t_total - 1
                                    or chunk_tile[ci + 1] != t)
                            if first and t % 4 == 0:
                                a_ps = pp.tile([128, 512], f32,
                                               space="PSUM", tag="a", bufs=2)
                            asl = slice((t % 4) * 128, (t % 4 + 1) * 128)
                            nc.tensor.matmul(
                                a_ps[:, asl], sel[:],
                                m1n[:, (k - g0) * 128:(k - g0 + 1) * 128],
                                start=first, stop=last)
                            if last:
                                if t % 4 == 0:
                                    ast4 = wp.tile([128, 4, 128], b16,
                                                   tag="ast", name=f"ast_{li}_{t}")
                                nc.vector.tensor_scalar_mul(
                                    ast4[:, t % 4, :], a_ps[:, asl],
                                    c_ivdn[:, t:t + 1])
                                if t % 4 == 3:
                                    half = t // HALF
                                    hr = (t - 3 - half * HALF) * 128
                                    nc.sync.dma_start(
                                        a_h[half][hr:hr + 512, :].rearrange(
                                            "(k p) f -> p k f", p=128),
                                        ast4[:])
                            if last and t == HALF - 1:
                                nc.gpsimd.collective_compute(
                                    "AllReduce", mybir.AluOpType.add,
                                    replica_groups=[list(range(NCORES))],
                                    ins=[a_h[0].opt()], outs=[a_r[0].opt()])
                nc.gpsimd.collective_compute(
                    "AllReduce", mybir.AluOpType.add,
                    replica_groups=[list(range(NCORES))],
                    ins=[a_h[1].opt()], outs=[a_r[1].opt()])

                # ---- node phase (feature-major, replicated, fp32) ----
                ps = pp.tile([128, 128], f32, space="PSUM", tag="p1ps", bufs=2)
                nc.tensor.matmul(ps[:], w["w2t"][:], w["wu1_bot"][:],
                                 start=True, stop=True)
                w2up = wp.tile([128, 128], b16, tag="w2up")
                nc.vector.tensor_copy(w2up[:], ps[:])
                for j in range(NCH):
                    cols = slice(j * 512, (j + 1) * 512)
                    half = j // (NCH // 2)
                    hcols = slice((j - half * (NCH // 2)) * 512,
                                  (j - half * (NCH // 2) + 1) * 512)
                    at = sp.tile([128, 512], b16, tag="ain")
                    nc.sync.dma_start(
                        at[:],
                        a_r[half][hcols, :],
                        transpose=True)
                    u1 = pp.tile([128, 512], f32, space="PSUM", tag="q", bufs=4)
                    nc.tensor.matmul(u1[:], w["wu1_top"][:], h_bf[:, cols],
                                     start=True, stop=False)
                    nc.tensor.matmul(u1[:], w2up[:], at[:],
                                     start=False, stop=not has_isolated)
                    if has_isolated:
                        nc.tensor.matmul(u1[:], w["c_row"][:],
                                         c_mask[:, cols],
                                         start=False, stop=True)
                    z1 = wp.tile([128, 512], b16, tag="z1")
                    nc.scalar.activation(z1[:], u1[:], Relu,
                                         bias=w["bu1_eff"][:])
                    u2 = pp.tile([128, 512], f32, space="PSUM", tag="a", bufs=2)
                    nc.tensor.matmul(u2[:], w["wu2"][:], z1[:],
                                     start=True, stop=True)
                    hn = wp.tile([128, 512], f32, tag="hn")
                    nc.vector.tensor_add(hn[:], h_fm[:, cols], u2[:])
                    nc.scalar.activation(h_fm[:, cols], hn[:], Relu,
                                         bias=w["bu2"][:])
                    if li == 0:
                        nc.vector.tensor_copy(h_bf[:, cols], h_fm[:, cols])

            # ---- prediction head (fp32) ----
            for j in range(NCH):
                cols = slice(j * 512, (j + 1) * 512)
                zp_ps = pp.tile([64, 512], f32, space="PSUM", tag="q", bufs=4)
                nc.tensor.matmul(zp_ps[:], c_wp1[:], h_fm[:, cols],
                                 start=True, stop=True)
                zp = wp.tile([64, 512], f32, tag="zp")
                nc.scalar.activation(zp[:], zp_ps[:], Relu, bias=c_bp1[:])
                y_ps = pp.tile([1, 512], f32, space="PSUM", tag="a", bufs=2)
                nc.tensor.matmul(y_ps[:], c_wp2[:], zp[:],
                                 start=True, stop=True)
                yt = wp.tile([1, 512], f32, tag="yt")
                nc.vector.tensor_scalar_add(yt[:], y_ps[:], c_bp2[:])
                nc.sync.dma_start(y_out[:, cols], yt[:])

    nc.compile()
    return nc


_CACHE = {}


def _get_program(meta_key, meta):
    if meta_key not in _CACHE:
        _CACHE[meta_key] = _build(meta)
    return _CACHE[meta_key]


def kernel(x, edge_index, edge_attr, edge_mark, deg_out, params,
           _trace=False):
    from concourse.bass_utils import run_bass_kernel_spmd

    in_maps, meta = _prep(x, edge_index, edge_attr, edge_mark, deg_out, params)
    meta_key = (meta["t_total"], tuple(meta["chunk_tile"]),
                meta["has_isolated"])
    nc = _get_program(meta_key, meta)

    kwargs = {}
    if _trace:
        # antenv.axon_hooks is absent in this image; provide the ctypes hook.
        try:
            import antenv.axon_hooks  # noqa: F401
        except ImportError:
            from trn_agent_boot.trn_boot import _ntff_profile_via_ctypes
            m = types.ModuleType("antenv.axon_hooks")
            m.get_axon_ntff_profile_hook = lambda: _ntff_profile_via_ctypes(
                "/opt/axon/libaxon_pjrt.so")
            sys.modules["antenv.axon_hooks"] = m
        kwargs["trace"] = True

    res = run_bass_kernel_spmd(nc, in_maps, list(range(NCORES)), **kwargs)
    y = np.asarray(res.results[0]["y"]).reshape(-1)[:N].astype(np.float32)
    if _trace:
        kernel._last_exec_time_ns = res.exec_time_ns
        kernel._last_results = res
    return y
